# revision 1
# baseline (speedup 1.0000x reference)
"""Trainium2 Bass kernel for nn_BinarizedRNN.

Math: the reference's output is out[t] = sign(hidden_t) @ sign(Wo).T where
hidden feeds the next step only through sign(hidden_t).  With l1,l2 > 0 the
SignSensitiveBatchNorm factor (s*l1 + (1-s)*l2)/sqrt(var+eps) is strictly
positive, so it never changes any sign; with sign(Wh) == I the recurrent
matmul is the identity.  The whole net collapses to

    q_t = (u'_t >= p_{t-1}),  p_t = q_t * (-2*g_{t+1}),   (elementwise)
    u'_t = x_t @ sign(Wi).T - g_t                         (one big matmul)
    out_t = (2*q_t - 1) @ sign(Wo).T

which maps to: one K=786-augmented matmul (hi/lo bf16 split, fp32-accurate),
a DVE tensor_tensor_scan(is_ge, mult) along time for the sign recurrence,
and an exact bf16 matmul for the output.  Data-parallel over B across 8
cores; no collectives needed (the batch-variance is provably inert).

Chain layout: rows are ordered (b, c) with c = 0 a reset column (u' = +BIG,
d1 = -g_1) so 4 independent b-chains of length 65 pack into one 260-column
r-tile and a single scan instruction handles all of them.
"""
import os
import numpy as np
import ml_dtypes

T, B, IN, H, OUT = 64, 256, 784, 2048, 256
EPS = 1e-5
NCORES = 8
BS = B // NCORES        # 32 batch rows per core
KAUG = IN + 2           # +g row, +reset row
CH = T + 1              # 65-column chains (reset + 64 steps)
NB = 4                  # b-chains per r-tile
RT = NB * CH            # 260
NRT = BS // NB          # 8 r-tiles per core
NHT = H // 128          # 16
NO = OUT // 128         # 2
RCOLS = BS * CH         # 2080 total row-columns per core
BIG = 1e9

# k-chunking of the 786-long contraction dim
KCHUNKS = []
_k0 = 0
while _k0 < KAUG:
    kn = min(128, KAUG - _k0)
    KCHUNKS.append((_k0, kn))
    _k0 += kn
KC = len(KCHUNKS)       # 7

KAUG2 = 2 * KAUG        # hilo2: hi rows stacked over lo rows
KCHUNKS2 = []
_k0 = 0
while _k0 < KAUG2:
    kn = min(128, KAUG2 - _k0)
    KCHUNKS2.append((_k0, kn))
    _k0 += kn

_CACHE = {}


def _build(mode: str, iters: int = 1):
    """Build the SPMD Bacc module. mode in {"hilo", "fp32"}."""
    import concourse.bacc as bacc
    import concourse.mybir as mybir
    import concourse.tile as tile

    f32 = mybir.dt.float32
    f32r = mybir.dt.float32r
    bf16 = mybir.dt.bfloat16

    nc = bacc.Bacc(
        "TRN2", target_bir_lowering=False, debug=False, num_devices=NCORES
    )

    if mode == "hilo2":
        xs_d = nc.dram_tensor("xs", [KAUG2, RCOLS], bf16, kind="ExternalInput")
        wi_d = nc.dram_tensor("wi", [KAUG, H], bf16, kind="ExternalInput")
    elif mode == "hilo":
        xhi_d = nc.dram_tensor("xhi", [KAUG, RCOLS], bf16, kind="ExternalInput")
        xlo_d = nc.dram_tensor("xlo", [KAUG, RCOLS], bf16, kind="ExternalInput")
        wi_d = nc.dram_tensor("wi", [KAUG, H], bf16, kind="ExternalInput")
    else:
        xt_d = nc.dram_tensor("xt", [KAUG, RCOLS], f32, kind="ExternalInput")
        wi_d = nc.dram_tensor("wi", [KAUG, H], f32, kind="ExternalInput")
    sb_dt = {"hilo": bf16, "hilo2": bf16, "fp32": f32, "fp32r": f32r}[mode]
    wo_d = nc.dram_tensor("wo", [H, OUT], bf16, kind="ExternalInput")
    d1_d = nc.dram_tensor("d1", [128, RT], f32, kind="ExternalInput")
    outt_d = nc.dram_tensor("outt", [OUT, BS * T], f32, kind="ExternalOutput")


    with tile.TileContext(nc) as tc:
        import contextlib
        with (
            tc.tile_pool(name="xw", bufs=1) as xw,
            tc.tile_pool(name="ppool", bufs=20) as ppool,
            tc.tile_pool(name="stage", bufs=4) as stage,
            tc.tile_pool(name="ps1", bufs=6, space="PSUM") as ps1,
            tc.tile_pool(name="ps2", bufs=2, space="PSUM") as ps2,
            (tc.For_i(0, iters, 1) if iters > 1 else contextlib.nullcontext()),
        ):
            # resident inputs
            w_tiles = []
            x_tiles = []  # list of tuples (per pass)
            if mode == "hilo2":
                for ci, (k0, kn) in enumerate(KCHUNKS2):
                    wt = xw.tile([kn, H], bf16, tag=f"w{ci}")
                    # weight rows repeat with period KAUG (hi and lo share W)
                    a0 = k0 % KAUG
                    n1 = min(kn, KAUG - a0)
                    nc.sync.dma_start(wt[:n1, :], wi_d[a0 : a0 + n1, :])
                    if n1 < kn:
                        nc.sync.dma_start(wt[n1:kn, :], wi_d[0 : kn - n1, :])
                    w_tiles.append(wt)
                    xt_ = xw.tile([kn, RCOLS], bf16, tag=f"xs{ci}")
                    nc.sync.dma_start(xt_[:], xs_d[k0 : k0 + kn, :])
                    x_tiles.append((xt_,))
            for ci, (k0, kn) in enumerate(KCHUNKS if mode != "hilo2" else []):
                wt = xw.tile([kn, H], sb_dt, tag=f"w{ci}")
                if mode == "fp32r":
                    nc.gpsimd.dma_start(wt[:], wi_d[k0 : k0 + kn, :])
                else:
                    nc.sync.dma_start(wt[:], wi_d[k0 : k0 + kn, :])
                w_tiles.append(wt)
                if mode == "hilo":
                    xh = xw.tile([kn, RCOLS], bf16, tag=f"xh{ci}")
                    xl = xw.tile([kn, RCOLS], bf16, tag=f"xl{ci}")
                    nc.sync.dma_start(xh[:], xhi_d[k0 : k0 + kn, :])
                    nc.sync.dma_start(xl[:], xlo_d[k0 : k0 + kn, :])
                    x_tiles.append((xh, xl))
                elif mode == "fp32":
                    xf = xw.tile([kn, RCOLS], f32, tag=f"xf{ci}")
                    nc.sync.dma_start(xf[:], xt_d[k0 : k0 + kn, :])
                    x_tiles.append((xf,))
                else:
                    xf = xw.tile([kn, RCOLS], f32r, tag=f"xr{ci}")
                    nc.gpsimd.dma_start(xf[:], xt_d[k0 : k0 + kn, :])
                    x_tiles.append((xf,))
            wo_t = xw.tile([128, NHT, OUT], bf16, tag="wo")
            nc.sync.dma_start(wo_t[:], wo_d.rearrange("(c p) o -> p c o", p=128))
            d1_t = xw.tile([128, RT], f32, tag="d1")
            nc.sync.dma_start(d1_t[:], d1_d[:])

            n_pass = len(x_tiles[0])
            n_mm = len(w_tiles) * n_pass
            if os.environ.get("BASS_NN_STRUCT", "v1") == "v2":
                # v2: ht-pairs with k-outermost (PE consumes X chunks as DMA
                # delivers them -> no cold-start stall) + incremental output
                # matmul accumulation (no end tail).  GRP fixed at 2.
                GRP, HTP = 2, 2
                for g in range(NRT // GRP):
                    rts = list(range(g * GRP, (g + 1) * GRP))
                    p_tiles = []
                    po = {}
                    for hp in range(NHT // HTP):
                        pss = [
                            [
                                ps1.tile([128, RT], f32, tag="mm1",
                                         name=f"ps_{g}_{hp}_{a}_{j}")
                                for j in range(GRP)
                            ]
                            for a in range(HTP)
                        ]
                        for i, (ci, xp) in enumerate(
                            (ci, xp)
                            for ci in range(len(w_tiles))
                            for xp in range(n_pass)
                        ):
                            for a in range(HTP):
                                ht = hp * HTP + a
                                for j, rt in enumerate(rts):
                                    nc.tensor.matmul(
                                        pss[a][j][:],
                                        w_tiles[ci][:, ht * 128 : (ht + 1) * 128],
                                        x_tiles[ci][xp][:, rt * RT : (rt + 1) * RT],
                                        start=(i == 0),
                                        stop=(i == n_mm - 1),
                                    )
                        for a in range(HTP):
                            p = ppool.tile([128, GRP * NB, CH], bf16, tag="p",
                                           name=f"p_{g}_{hp}_{a}")
                            for j in range(GRP):
                                nc.vector.tensor_tensor_scan(
                                    p[:, j * NB : (j + 1) * NB, :].rearrange(
                                        "p a b -> p (a b)"
                                    ),
                                    pss[a][j][:],
                                    d1_t[:],
                                    0.0,
                                    mybir.AluOpType.is_ge,
                                    mybir.AluOpType.mult,
                                )
                            p_tiles.append(p)
                        # incremental output-matmul accumulation over ht
                        for o in range(NO):
                            if hp == 0:
                                po[o] = ps2.tile([128, GRP * NB * T], f32,
                                                 tag="mm2", name=f"po_{g}_{o}")
                            for a in range(HTP):
                                ht = hp * HTP + a
                                nc.tensor.matmul(
                                    po[o][:],
                                    wo_t[:, ht, o * 128 : (o + 1) * 128],
                                    p_tiles[ht][:, :, 1:],
                                    start=(ht == 0),
                                    stop=(ht == NHT - 1),
                                )
                    for o in range(NO):
                        st = stage.tile([128, GRP * NB * T], f32, tag="st",
                                        name=f"st_{g}_{o}")
                        nc.vector.tensor_copy(st[:], po[o][:])
                        col = g * GRP * NB * T
                        nc.sync.dma_start(
                            outt_d[o * 128 : (o + 1) * 128, col : col + GRP * NB * T],
                            st[:],
                        )
            else:
                GRP = int(os.environ.get("BASS_NN_GRP", "2"))  # r-tiles per group
                n_mm = KC * n_pass
                for g in range(NRT // GRP):
                    rts = list(range(g * GRP, (g + 1) * GRP))
                    p_tiles = []              # one [128, GRP*NB, CH] tile per ht
                    for ht in range(NHT):
                        pss = [ps1.tile([128, RT], f32, tag="mm1", name=f"ps_{g}_{ht}_{j}") for j in range(len(rts))]
                        for i, (ci, xp) in enumerate(
                            (ci, xp)
                            for ci in range(len(w_tiles))
                            for xp in range(n_pass)
                        ):
                            for j, rt in enumerate(rts):
                                nc.tensor.matmul(
                                    pss[j][:],
                                    w_tiles[ci][:, ht * 128 : (ht + 1) * 128],
                                    x_tiles[ci][xp][:, rt * RT : (rt + 1) * RT],
                                    start=(i == 0),
                                    stop=(i == n_mm - 1),
                                )
                        p = ppool.tile([128, GRP * NB, CH], bf16, tag="p")
                        ablate = os.environ.get("BASS_NN_ABLATE", "none")
                        for j in range(GRP):
                            pv = p[:, j * NB : (j + 1) * NB, :].rearrange(
                                "p a b -> p (a b)"
                            )
                            if ablate == "noscan":
                                nc.vector.tensor_copy(pv, pss[j][:])
                            else:
                                nc.vector.tensor_tensor_scan(
                                    pv,
                                    pss[j][:],
                                    d1_t[:],
                                    0.0,
                                    mybir.AluOpType.is_ge,
                                    mybir.AluOpType.mult,
                                )
                        p_tiles.append(p)
                    # output matmuls: rt-pairs -> N=512, skip reset columns
                    PW = 2 if GRP % 2 == 0 else 1
                    for pr in range(0 if os.environ.get("BASS_NN_ABLATE") == "nomm2" else GRP // PW):
                        for o in range(NO):
                            po = ps2.tile([128, PW * NB * T], f32, tag="mm2")
                            for ht in range(NHT):
                                nc.tensor.matmul(
                                    po[:],
                                    wo_t[:, ht, o * 128 : (o + 1) * 128],
                                    p_tiles[ht][:, PW * NB * pr : PW * NB * (pr + 1), 1:],
                                    start=(ht == 0),
                                    stop=(ht == NHT - 1),
                                )
                            st = stage.tile([128, PW * NB * T], f32, tag="st")
                            nc.vector.tensor_copy(st[:], po[:])
                            col = (g * GRP + PW * pr) * NB * T
                            nc.sync.dma_start(
                                outt_d[o * 128 : (o + 1) * 128, col : col + PW * NB * T],
                                st[:],
                            )

    nc.compile()
    return nc


def _get_module(mode, iters=1):
    key = (mode, iters, os.environ.get("BASS_NN_GRP", "2"),
           os.environ.get("BASS_NN_ABLATE", "none"),
           os.environ.get("BASS_NN_STRUCT", "v1"))
    if key not in _CACHE:
        _CACHE[key] = _build(mode, iters)
    return _CACHE[key]


def _fallback_numpy(x, Wi, Wh, Wo, gates, l1, l2):
    """Direct fp32 replication of the reference for degenerate inputs."""
    Wi_b = np.sign(Wi)
    Wh_b = np.sign(Wh)
    Wo_b = np.sign(Wo)
    Bn, Hn = x.shape[1], Wi.shape[0]
    h = np.zeros((Bn, Hn), dtype=np.float32)
    outs = []
    for t in range(x.shape[0]):
        hidden = x[t] @ Wi_b.T + gates[t] * (np.sign(h) @ Wh_b.T)
        hidden = np.clip(hidden, -1.0, 1.0)
        var = hidden.var(axis=0, ddof=1, keepdims=True)
        bottom = np.sqrt(var + EPS)
        s = 1.0 / (1.0 + np.exp(-10.0 * hidden))
        hidden = (hidden * s * l1 + hidden * (1.0 - s) * l2) / bottom
        outs.append(np.sign(hidden) @ Wo_b.T)
        h = hidden
    return np.stack(outs).astype(np.float32)


def _prep_in_maps(x, gates, wi_aug, wo_arr, d1, mode):
    """Per-core X^T with augmentation rows and reset columns: [KAUG, BS*CH].
    Column order: (b, c) with c=0 reset, c>=1 -> timestep c-1."""
    in_maps = []
    if mode == "hilo":
        wi_hi = wi_aug.astype(ml_dtypes.bfloat16)
        wi_lo = (wi_aug - wi_hi.astype(np.float32)).astype(ml_dtypes.bfloat16)
        # weights are +-1/0 and small aug values: hi is exact, lo == 0
        assert np.all(wi_lo.astype(np.float32) == 0.0)
    for c in range(NCORES):
        xs = x[:, c * BS : (c + 1) * BS, :]             # [T, BS, IN]
        xa = np.zeros((KAUG, BS, CH), dtype=np.float32)
        xa[:IN, :, 1:] = xs.transpose(2, 1, 0)          # [IN, BS, T]
        xa[IN, :, 1:] = gates[None, :]                  # g_t row
        xa[IN + 1, :, 0] = BIG                          # reset row
        xa = xa.reshape(KAUG, RCOLS)
        m = {"wo": wo_arr, "d1": d1}
        if mode == "hilo2":
            xhi = xa.astype(ml_dtypes.bfloat16)
            xlo = (xa - xhi.astype(np.float32)).astype(ml_dtypes.bfloat16)
            m["xs"] = np.vstack([xhi, xlo])
            m["wi"] = wi_aug.astype(ml_dtypes.bfloat16)
        elif mode == "hilo":
            xhi = xa.astype(ml_dtypes.bfloat16)
            xlo = (xa - xhi.astype(np.float32)).astype(ml_dtypes.bfloat16)
            m["xhi"] = xhi
            m["xlo"] = xlo
            m["wi"] = wi_hi
        else:
            m["xt"] = xa
            m["wi"] = wi_aug
        in_maps.append(m)
    return in_maps


LAST_RESULTS = None


def kernel(x, Wi, Wh, Wo, gates, l1, l2):
    global LAST_RESULTS
    x = np.asarray(x, dtype=np.float32)
    Wi = np.asarray(Wi, dtype=np.float32)
    Wh = np.asarray(Wh, dtype=np.float32)
    Wo = np.asarray(Wo, dtype=np.float32)
    gates = np.asarray(gates, dtype=np.float32)
    l1 = np.asarray(l1, dtype=np.float32)
    l2 = np.asarray(l2, dtype=np.float32)

    fast = (
        x.shape == (T, B, IN)
        and np.all(l1 > 0)
        and np.all(l2 > 0)
        and np.array_equal(np.sign(Wh), np.eye(H, dtype=np.float32))
        and np.all(gates[1:] != 0)
    )
    if not fast:
        return _fallback_numpy(x, Wi, Wh, Wo, gates, l1, l2)

    from concourse.bass_utils import run_bass_kernel_spmd

    mode = os.environ.get("BASS_NN_MODE", "hilo")
    nc = _get_module(mode)

    Wi_b = np.sign(Wi)                      # [H, IN]
    Wo_b = np.sign(Wo)                      # [OUT, H]
    colsum = Wo_b.sum(axis=1)               # [OUT]

    # augmented, transposed input-weights: [KAUG, H]
    wi_aug = np.empty((KAUG, H), dtype=np.float32)
    wi_aug[:IN] = Wi_b.T
    wi_aug[IN] = -1.0                       # g row
    wi_aug[IN + 1] = 1.0                    # reset row
    wo_arr = np.ascontiguousarray(Wo_b.T).astype(ml_dtypes.bfloat16)  # [H, OUT]

    # d1 per chain column: c=0 -> -g_1 ; c=1..63 -> -2*g_{c+1} ; c=64 -> -2
    gamma = np.empty(T, dtype=np.float32)   # scale for output recovery
    gamma[: T - 1] = gates[1:]
    gamma[T - 1] = 1.0
    dd = np.empty(CH, dtype=np.float32)
    dd[0] = -gates[0]
    dd[1:] = -2.0 * gamma
    d1 = np.tile(np.tile(dd, NB)[None, :], (128, 1)).astype(np.float32)

    in_maps = _prep_in_maps(x, gates, wi_aug, wo_arr, d1, mode)
    res = run_bass_kernel_spmd(nc, in_maps, core_ids=list(range(NCORES)))
    LAST_RESULTS = res

    out = np.empty((T, B, OUT), dtype=np.float32)
    inv_gamma = (1.0 / gamma).astype(np.float32)        # [T]
    for c in range(NCORES):
        ot = res.results[c]["outt"].reshape(OUT, BS, T)
        # out[t, b, o] = -ot[o, b, t]/gamma[t] - colsum[o]
        out[:, c * BS : (c + 1) * BS, :] = (
            -ot.transpose(2, 1, 0) * inv_gamma[:, None, None]
            - colsum[None, None, :]
        )
    return out



# revision 7
# speedup vs baseline: 1.4231x; 1.4231x over previous
"""Trainium2 Bass kernel for nn_BinarizedRNN.

Math: the reference's output is out[t] = sign(hidden_t) @ sign(Wo).T where
hidden feeds the next step only through sign(hidden_t).  With l1,l2 > 0 the
SignSensitiveBatchNorm factor (s*l1 + (1-s)*l2)/sqrt(var+eps) is strictly
positive, so it never changes any sign; with sign(Wh) == I the recurrent
matmul is the identity.  The whole net collapses to

    q_t = (u'_t >= p_{t-1}),  p_t = q_t * (-2*g_{t+1}),   (elementwise)
    u'_t = x_t @ sign(Wi).T - g_t                         (one big matmul)
    out_t = (2*q_t - 1) @ sign(Wo).T

which maps to: one K=786-augmented matmul (hi/lo bf16 split, fp32-accurate),
a DVE tensor_tensor_scan(is_ge, mult) along time for the sign recurrence,
and an exact bf16 matmul for the output.  Data-parallel over B across 8
cores; no collectives needed (the batch-variance is provably inert).

Chain layout: rows are ordered (b, c) with c = 0 a reset column (u' = +BIG,
d1 = -g_1) so 4 independent b-chains of length 65 pack into one 260-column
r-tile and a single scan instruction handles all of them.
"""
import os
import numpy as np
import ml_dtypes

T, B, IN, H, OUT = 64, 256, 784, 2048, 256
EPS = 1e-5
NCORES = 8
BS = B // NCORES        # 32 batch rows per core
KAUG = IN + 2           # +g row, +reset row
CH = T + 1              # 65-column chains (reset + 64 steps)
NB = 4                  # b-chains per r-tile
RT = NB * CH            # 260
NRT = BS // NB          # 8 r-tiles per core
NHT = H // 128          # 16
NO = OUT // 128         # 2
RCOLS = BS * CH         # 2080 total row-columns per core
BIG = 1e9

# k-chunking of the 786-long contraction dim
KCHUNKS = []
_k0 = 0
while _k0 < KAUG:
    kn = min(128, KAUG - _k0)
    KCHUNKS.append((_k0, kn))
    _k0 += kn
KC = len(KCHUNKS)       # 7

KAUG2 = 2 * KAUG        # hilo2: hi rows stacked over lo rows
KCHUNKS2 = []
_k0 = 0
while _k0 < KAUG2:
    kn = min(128, KAUG2 - _k0)
    KCHUNKS2.append((_k0, kn))
    _k0 += kn

_CACHE = {}

# ---- fp8 DoubleRow mode constants ----
# Stacked-K contraction: section A = [784 x-rows (fp8 hi) ; g row ; reset row],
# B = fp8((x-hi)*64) with weights Wi_b*2^-6, C = fp8((x-hi-mid/64)*64) same
# weights.  One fp32-PSUM accumulation over all sections at fp8 DoubleRow rate
# (0.5 cyc/col).  Verified numerically: rel_err ~ 4.6e-3 vs exact.
KA = IN + 2                  # 786
KTOT = KA + 2 * IN           # 2354
NKP = 10                     # k-pairs of 256 (KTOT padded to 2560)
KPAD = NKP * 256
# psum tiles: one bank each (<=512 f32); chain-aligned so scans read a whole
# tile.  2080 = 4*455 + 260 (7,7,7,7,4 chains).
PTILES = [(0, 455), (455, 455), (910, 455), (1365, 455), (1820, 260)]
BIG8 = 128.0                 # reset magnitude within fp8e4 range


def _mm_wins(n):
    return [(0, 256), (256, n - 256)] if n > 256 else [(0, n)]


def _build_fp8(iters: int = 1):
    import concourse.bacc as bacc
    import concourse.mybir as mybir
    import concourse.tile as tile
    import contextlib

    f32 = mybir.dt.float32
    fp8 = mybir.dt.float8e4
    DR = mybir.MatmulPerfMode.DoubleRow

    nc = bacc.Bacc(
        "TRN2", target_bir_lowering=False, debug=False, num_devices=NCORES
    )

    xs_d = nc.dram_tensor("xs", [128, NKP, 2, RCOLS], fp8, kind="ExternalInput")
    ws_d = nc.dram_tensor("ws", [128, NHT, NKP, 2, 128], fp8, kind="ExternalInput")
    wo_d = nc.dram_tensor("wo", [128, NHT // 2, 2, OUT], fp8, kind="ExternalInput")
    d1_d = nc.dram_tensor("d1", [128, RCOLS], f32, kind="ExternalInput")
    outt_d = nc.dram_tensor("outt", [OUT, RCOLS], f32, kind="ExternalOutput")

    with tile.TileContext(nc) as tc:
        with (
            tc.tile_pool(name="xw", bufs=1) as xw,
            tc.tile_pool(name="ppool", bufs=1) as ppool,
            tc.tile_pool(name="stage", bufs=3) as stage,
            tc.tile_pool(name="ps1", bufs=6, space="PSUM") as ps1,
            tc.tile_pool(name="ps2", bufs=2, space="PSUM") as ps2,
            (tc.For_i(0, iters, 1) if iters > 1 else contextlib.nullcontext()),
        ):
            d1_t = xw.tile([128, RCOLS], f32, tag="d1")
            nc.sync.dma_start(d1_t[:], d1_d[:])
            ws_t = xw.tile([128, NHT, NKP, 2, 128], fp8, tag="ws")
            xs_t = xw.tile([128, NKP, 2, RCOLS], fp8, tag="xs")
            # interleave: first compute tile needs ws[ht0] + xs chunk 0 only
            nc.sync.dma_start(ws_t[:, 0], ws_d[:, 0])
            for kp in range(NKP):
                nc.sync.dma_start(xs_t[:, kp, :, 0:520], xs_d[:, kp, :, 0:520])
            for ht in range(1, NHT):
                nc.sync.dma_start(ws_t[:, ht], ws_d[:, ht])
            for cc in range(1, 4):                   # column chunks of 520
                for kp in range(NKP):
                    nc.sync.dma_start(
                        xs_t[:, kp, :, cc * 520 : (cc + 1) * 520],
                        xs_d[:, kp, :, cc * 520 : (cc + 1) * 520],
                    )
            wo_t = xw.tile([128, NHT // 2, 2, OUT], fp8, tag="wo")
            nc.sync.dma_start(wo_t[:], wo_d[:])
            p_all = ppool.tile([128, NHT, RCOLS], fp8, tag="p")

            for ht in range(NHT):
                for (t0, tn) in PTILES:
                    ps = ps1.tile([128, tn], f32, tag="mm1",
                                  name=f"ps_{ht}_{t0}")
                    wins = _mm_wins(tn)
                    nw = len(wins)
                    for kp in range(NKP):
                        for wi, (w0, wn) in enumerate(wins):
                            nc.tensor.matmul(
                                ps[:, w0 : w0 + wn],
                                ws_t[:, ht, kp, :, :],
                                xs_t[:, kp, :, t0 + w0 : t0 + w0 + wn],
                                start=(kp == 0 and wi == 0),
                                stop=(kp == NKP - 1 and wi == nw - 1),
                                perf_mode=DR,
                            )
                    nc.vector.tensor_tensor_scan(
                        p_all[:, ht, t0 : t0 + tn],
                        ps[:],
                        d1_t[:, t0 : t0 + tn],
                        0.0,
                        mybir.AluOpType.is_ge,
                        mybir.AluOpType.mult,
                    )

            for cw in range(0, RCOLS, 512):
                cn = min(512, RCOLS - cw)
                for o in range(NO):
                    po = ps2.tile([128, cn], f32, tag="mm2",
                                  name=f"po_{cw}_{o}")
                    wins = _mm_wins(cn)
                    nw = len(wins)
                    for pr in range(NHT // 2):
                        for wi, (w0, wn) in enumerate(wins):
                            nc.tensor.matmul(
                                po[:, w0 : w0 + wn],
                                wo_t[:, pr, :, o * 128 : (o + 1) * 128],
                                p_all[:, 2 * pr : 2 * pr + 2, cw + w0 : cw + w0 + wn],
                                start=(pr == 0 and wi == 0),
                                stop=(pr == NHT // 2 - 1 and wi == nw - 1),
                                perf_mode=DR,
                            )
                    st = stage.tile([128, cn], f32, tag="st",
                                    name=f"st_{cw}_{o}")
                    nc.vector.tensor_copy(st[:], po[:])
                    nc.sync.dma_start(
                        outt_d[o * 128 : (o + 1) * 128, cw : cw + cn],
                        st[:],
                    )

    nc.compile()
    return nc


def _prep_fp8_in_maps(x, gates, Wi_b, Wo_b, d1):
    """Build per-core fp8 stacked inputs."""
    f8 = ml_dtypes.float8_e4m3
    w_stack = np.zeros((KPAD, H), dtype=np.float32)
    w_stack[:IN] = Wi_b.T
    w_stack[IN] = -1.0
    w_stack[IN + 1] = 1.0
    w_stack[KA : KA + IN] = Wi_b.T * 2.0**-6
    w_stack[KA + IN : KTOT] = Wi_b.T * 2.0**-6
    ws = np.ascontiguousarray(
        w_stack.reshape(NKP, 2, 128, NHT, 128).transpose(2, 3, 0, 1, 4)
    ).astype(f8)
    wo = np.ascontiguousarray(
        Wo_b.T.reshape(NHT // 2, 2, 128, OUT).transpose(2, 0, 1, 3)
    ).astype(f8)
    in_maps = []
    for c in range(NCORES):
        xa = np.zeros((KA, BS, CH), dtype=np.float32)
        xa[:IN, :, 1:] = x[:, c * BS : (c + 1) * BS, :].transpose(2, 1, 0)
        xa[IN, :, 1:] = gates[None, :]
        xa[IN + 1, :, 0] = BIG8
        xa = xa.reshape(KA, RCOLS)
        hi = xa.astype(f8)
        hif = hi.astype(np.float32)
        mid = ((xa[:IN] - hif[:IN]) * 64.0).astype(f8)
        lo = ((xa[:IN] - hif[:IN] - mid.astype(np.float32) / 64.0) * 64.0).astype(f8)
        x_stack = np.zeros((KPAD, RCOLS), dtype=f8)
        x_stack[:KA] = hi
        x_stack[KA : KA + IN] = mid
        x_stack[KA + IN : KTOT] = lo
        xs = np.ascontiguousarray(
            x_stack.reshape(NKP, 2, 128, RCOLS).transpose(2, 0, 1, 3)
        )
        in_maps.append({"xs": xs, "ws": ws, "wo": wo, "d1": d1})
    return in_maps


def _build(mode: str, iters: int = 1):
    """Build the SPMD Bacc module. mode in {"hilo", "fp32"}."""
    import concourse.bacc as bacc
    import concourse.mybir as mybir
    import concourse.tile as tile

    f32 = mybir.dt.float32
    f32r = mybir.dt.float32r
    bf16 = mybir.dt.bfloat16

    nc = bacc.Bacc(
        "TRN2", target_bir_lowering=False, debug=False, num_devices=NCORES
    )

    if mode == "hilo2":
        xs_d = nc.dram_tensor("xs", [KAUG2, RCOLS], bf16, kind="ExternalInput")
        wi_d = nc.dram_tensor("wi", [KAUG, H], bf16, kind="ExternalInput")
    elif mode == "hilo":
        xhi_d = nc.dram_tensor("xhi", [KAUG, RCOLS], bf16, kind="ExternalInput")
        xlo_d = nc.dram_tensor("xlo", [KAUG, RCOLS], bf16, kind="ExternalInput")
        wi_d = nc.dram_tensor("wi", [KAUG, H], bf16, kind="ExternalInput")
    else:
        xt_d = nc.dram_tensor("xt", [KAUG, RCOLS], f32, kind="ExternalInput")
        wi_d = nc.dram_tensor("wi", [KAUG, H], f32, kind="ExternalInput")
    sb_dt = {"hilo": bf16, "hilo2": bf16, "fp32": f32, "fp32r": f32r}[mode]
    wo_d = nc.dram_tensor("wo", [H, OUT], bf16, kind="ExternalInput")
    d1_d = nc.dram_tensor("d1", [128, RT], f32, kind="ExternalInput")
    outt_d = nc.dram_tensor("outt", [OUT, BS * T], f32, kind="ExternalOutput")


    with tile.TileContext(nc) as tc:
        import contextlib
        with (
            tc.tile_pool(name="xw", bufs=1) as xw,
            tc.tile_pool(name="ppool", bufs=20) as ppool,
            tc.tile_pool(name="stage", bufs=4) as stage,
            tc.tile_pool(name="ps1", bufs=6, space="PSUM") as ps1,
            tc.tile_pool(name="ps2", bufs=2, space="PSUM") as ps2,
            (tc.For_i(0, iters, 1) if iters > 1 else contextlib.nullcontext()),
        ):
            # resident inputs
            w_tiles = []
            x_tiles = []  # list of tuples (per pass)
            if mode == "hilo2":
                for ci, (k0, kn) in enumerate(KCHUNKS2):
                    wt = xw.tile([kn, H], bf16, tag=f"w{ci}")
                    # weight rows repeat with period KAUG (hi and lo share W)
                    a0 = k0 % KAUG
                    n1 = min(kn, KAUG - a0)
                    nc.sync.dma_start(wt[:n1, :], wi_d[a0 : a0 + n1, :])
                    if n1 < kn:
                        nc.sync.dma_start(wt[n1:kn, :], wi_d[0 : kn - n1, :])
                    w_tiles.append(wt)
                    xt_ = xw.tile([kn, RCOLS], bf16, tag=f"xs{ci}")
                    nc.sync.dma_start(xt_[:], xs_d[k0 : k0 + kn, :])
                    x_tiles.append((xt_,))
            for ci, (k0, kn) in enumerate(KCHUNKS if mode != "hilo2" else []):
                wt = xw.tile([kn, H], sb_dt, tag=f"w{ci}")
                if mode == "fp32r":
                    nc.gpsimd.dma_start(wt[:], wi_d[k0 : k0 + kn, :])
                else:
                    nc.sync.dma_start(wt[:], wi_d[k0 : k0 + kn, :])
                w_tiles.append(wt)
                if mode == "hilo":
                    xh = xw.tile([kn, RCOLS], bf16, tag=f"xh{ci}")
                    xl = xw.tile([kn, RCOLS], bf16, tag=f"xl{ci}")
                    nc.sync.dma_start(xh[:], xhi_d[k0 : k0 + kn, :])
                    nc.sync.dma_start(xl[:], xlo_d[k0 : k0 + kn, :])
                    x_tiles.append((xh, xl))
                elif mode == "fp32":
                    xf = xw.tile([kn, RCOLS], f32, tag=f"xf{ci}")
                    nc.sync.dma_start(xf[:], xt_d[k0 : k0 + kn, :])
                    x_tiles.append((xf,))
                else:
                    xf = xw.tile([kn, RCOLS], f32r, tag=f"xr{ci}")
                    nc.gpsimd.dma_start(xf[:], xt_d[k0 : k0 + kn, :])
                    x_tiles.append((xf,))
            wo_t = xw.tile([128, NHT, OUT], bf16, tag="wo")
            nc.sync.dma_start(wo_t[:], wo_d.rearrange("(c p) o -> p c o", p=128))
            d1_t = xw.tile([128, RT], f32, tag="d1")
            nc.sync.dma_start(d1_t[:], d1_d[:])

            n_pass = len(x_tiles[0])
            n_mm = len(w_tiles) * n_pass
            if os.environ.get("BASS_NN_STRUCT", "v1") == "v2":
                # v2: ht-pairs with k-outermost (PE consumes X chunks as DMA
                # delivers them -> no cold-start stall) + incremental output
                # matmul accumulation (no end tail).  GRP fixed at 2.
                GRP, HTP = 2, 2
                for g in range(NRT // GRP):
                    rts = list(range(g * GRP, (g + 1) * GRP))
                    p_tiles = []
                    po = {}
                    for hp in range(NHT // HTP):
                        pss = [
                            [
                                ps1.tile([128, RT], f32, tag="mm1",
                                         name=f"ps_{g}_{hp}_{a}_{j}")
                                for j in range(GRP)
                            ]
                            for a in range(HTP)
                        ]
                        for i, (ci, xp) in enumerate(
                            (ci, xp)
                            for ci in range(len(w_tiles))
                            for xp in range(n_pass)
                        ):
                            for a in range(HTP):
                                ht = hp * HTP + a
                                for j, rt in enumerate(rts):
                                    nc.tensor.matmul(
                                        pss[a][j][:],
                                        w_tiles[ci][:, ht * 128 : (ht + 1) * 128],
                                        x_tiles[ci][xp][:, rt * RT : (rt + 1) * RT],
                                        start=(i == 0),
                                        stop=(i == n_mm - 1),
                                    )
                        for a in range(HTP):
                            p = ppool.tile([128, GRP * NB, CH], bf16, tag="p",
                                           name=f"p_{g}_{hp}_{a}")
                            for j in range(GRP):
                                nc.vector.tensor_tensor_scan(
                                    p[:, j * NB : (j + 1) * NB, :].rearrange(
                                        "p a b -> p (a b)"
                                    ),
                                    pss[a][j][:],
                                    d1_t[:],
                                    0.0,
                                    mybir.AluOpType.is_ge,
                                    mybir.AluOpType.mult,
                                )
                            p_tiles.append(p)
                        # incremental output-matmul accumulation over ht
                        for o in range(NO):
                            if hp == 0:
                                po[o] = ps2.tile([128, GRP * NB * T], f32,
                                                 tag="mm2", name=f"po_{g}_{o}")
                            for a in range(HTP):
                                ht = hp * HTP + a
                                nc.tensor.matmul(
                                    po[o][:],
                                    wo_t[:, ht, o * 128 : (o + 1) * 128],
                                    p_tiles[ht][:, :, 1:],
                                    start=(ht == 0),
                                    stop=(ht == NHT - 1),
                                )
                    for o in range(NO):
                        st = stage.tile([128, GRP * NB * T], f32, tag="st",
                                        name=f"st_{g}_{o}")
                        nc.vector.tensor_copy(st[:], po[o][:])
                        col = g * GRP * NB * T
                        nc.sync.dma_start(
                            outt_d[o * 128 : (o + 1) * 128, col : col + GRP * NB * T],
                            st[:],
                        )
            else:
                GRP = int(os.environ.get("BASS_NN_GRP", "2"))  # r-tiles per group
                n_mm = KC * n_pass
                for g in range(NRT // GRP):
                    rts = list(range(g * GRP, (g + 1) * GRP))
                    p_tiles = []              # one [128, GRP*NB, CH] tile per ht
                    for ht in range(NHT):
                        pss = [ps1.tile([128, RT], f32, tag="mm1", name=f"ps_{g}_{ht}_{j}") for j in range(len(rts))]
                        for i, (ci, xp) in enumerate(
                            (ci, xp)
                            for ci in range(len(w_tiles))
                            for xp in range(n_pass)
                        ):
                            for j, rt in enumerate(rts):
                                nc.tensor.matmul(
                                    pss[j][:],
                                    w_tiles[ci][:, ht * 128 : (ht + 1) * 128],
                                    x_tiles[ci][xp][:, rt * RT : (rt + 1) * RT],
                                    start=(i == 0),
                                    stop=(i == n_mm - 1),
                                )
                        p = ppool.tile([128, GRP * NB, CH], bf16, tag="p")
                        ablate = os.environ.get("BASS_NN_ABLATE", "none")
                        for j in range(GRP):
                            pv = p[:, j * NB : (j + 1) * NB, :].rearrange(
                                "p a b -> p (a b)"
                            )
                            if ablate == "noscan":
                                nc.vector.tensor_copy(pv, pss[j][:])
                            else:
                                nc.vector.tensor_tensor_scan(
                                    pv,
                                    pss[j][:],
                                    d1_t[:],
                                    0.0,
                                    mybir.AluOpType.is_ge,
                                    mybir.AluOpType.mult,
                                )
                        p_tiles.append(p)
                    # output matmuls: rt-pairs -> N=512, skip reset columns
                    PW = 2 if GRP % 2 == 0 else 1
                    for pr in range(0 if os.environ.get("BASS_NN_ABLATE") == "nomm2" else GRP // PW):
                        for o in range(NO):
                            po = ps2.tile([128, PW * NB * T], f32, tag="mm2")
                            for ht in range(NHT):
                                nc.tensor.matmul(
                                    po[:],
                                    wo_t[:, ht, o * 128 : (o + 1) * 128],
                                    p_tiles[ht][:, PW * NB * pr : PW * NB * (pr + 1), 1:],
                                    start=(ht == 0),
                                    stop=(ht == NHT - 1),
                                )
                            st = stage.tile([128, PW * NB * T], f32, tag="st")
                            nc.vector.tensor_copy(st[:], po[:])
                            col = (g * GRP + PW * pr) * NB * T
                            nc.sync.dma_start(
                                outt_d[o * 128 : (o + 1) * 128, col : col + PW * NB * T],
                                st[:],
                            )

    nc.compile()
    return nc


def _get_module(mode, iters=1):
    key = (mode, iters, os.environ.get("BASS_NN_GRP", "2"),
           os.environ.get("BASS_NN_ABLATE", "none"),
           os.environ.get("BASS_NN_STRUCT", "v1"))
    if key not in _CACHE:
        _CACHE[key] = _build_fp8(iters) if mode == "fp8" else _build(mode, iters)
    return _CACHE[key]


def _fallback_numpy(x, Wi, Wh, Wo, gates, l1, l2):
    """Direct fp32 replication of the reference for degenerate inputs."""
    Wi_b = np.sign(Wi)
    Wh_b = np.sign(Wh)
    Wo_b = np.sign(Wo)
    Bn, Hn = x.shape[1], Wi.shape[0]
    h = np.zeros((Bn, Hn), dtype=np.float32)
    outs = []
    for t in range(x.shape[0]):
        hidden = x[t] @ Wi_b.T + gates[t] * (np.sign(h) @ Wh_b.T)
        hidden = np.clip(hidden, -1.0, 1.0)
        var = hidden.var(axis=0, ddof=1, keepdims=True)
        bottom = np.sqrt(var + EPS)
        s = 1.0 / (1.0 + np.exp(-10.0 * hidden))
        hidden = (hidden * s * l1 + hidden * (1.0 - s) * l2) / bottom
        outs.append(np.sign(hidden) @ Wo_b.T)
        h = hidden
    return np.stack(outs).astype(np.float32)


def _prep_in_maps(x, gates, wi_aug, wo_arr, d1, mode):
    """Per-core X^T with augmentation rows and reset columns: [KAUG, BS*CH].
    Column order: (b, c) with c=0 reset, c>=1 -> timestep c-1."""
    in_maps = []
    if mode == "hilo":
        wi_hi = wi_aug.astype(ml_dtypes.bfloat16)
        wi_lo = (wi_aug - wi_hi.astype(np.float32)).astype(ml_dtypes.bfloat16)
        # weights are +-1/0 and small aug values: hi is exact, lo == 0
        assert np.all(wi_lo.astype(np.float32) == 0.0)
    for c in range(NCORES):
        xs = x[:, c * BS : (c + 1) * BS, :]             # [T, BS, IN]
        xa = np.zeros((KAUG, BS, CH), dtype=np.float32)
        xa[:IN, :, 1:] = xs.transpose(2, 1, 0)          # [IN, BS, T]
        xa[IN, :, 1:] = gates[None, :]                  # g_t row
        xa[IN + 1, :, 0] = BIG                          # reset row
        xa = xa.reshape(KAUG, RCOLS)
        m = {"wo": wo_arr, "d1": d1}
        if mode == "hilo2":
            xhi = xa.astype(ml_dtypes.bfloat16)
            xlo = (xa - xhi.astype(np.float32)).astype(ml_dtypes.bfloat16)
            m["xs"] = np.vstack([xhi, xlo])
            m["wi"] = wi_aug.astype(ml_dtypes.bfloat16)
        elif mode == "hilo":
            xhi = xa.astype(ml_dtypes.bfloat16)
            xlo = (xa - xhi.astype(np.float32)).astype(ml_dtypes.bfloat16)
            m["xhi"] = xhi
            m["xlo"] = xlo
            m["wi"] = wi_hi
        else:
            m["xt"] = xa
            m["wi"] = wi_aug
        in_maps.append(m)
    return in_maps


LAST_RESULTS = None


def kernel(x, Wi, Wh, Wo, gates, l1, l2):
    global LAST_RESULTS
    x = np.asarray(x, dtype=np.float32)
    Wi = np.asarray(Wi, dtype=np.float32)
    Wh = np.asarray(Wh, dtype=np.float32)
    Wo = np.asarray(Wo, dtype=np.float32)
    gates = np.asarray(gates, dtype=np.float32)
    l1 = np.asarray(l1, dtype=np.float32)
    l2 = np.asarray(l2, dtype=np.float32)

    fast = (
        x.shape == (T, B, IN)
        and np.all(l1 > 0)
        and np.all(l2 > 0)
        and np.array_equal(np.sign(Wh), np.eye(H, dtype=np.float32))
        and np.all(gates[1:] != 0)
    )
    if not fast:
        return _fallback_numpy(x, Wi, Wh, Wo, gates, l1, l2)

    from concourse.bass_utils import run_bass_kernel_spmd

    mode = os.environ.get("BASS_NN_MODE", "fp8")

    Wi_b = np.sign(Wi)                      # [H, IN]
    Wo_b = np.sign(Wo)                      # [OUT, H]
    colsum = Wo_b.sum(axis=1)               # [OUT]

    if mode == "fp8":
        f8 = ml_dtypes.float8_e4m3
        gamma_ = np.empty(T, dtype=np.float32)
        gamma_[: T - 1] = gates[1:]
        gamma_[T - 1] = 1.0
        dd_ = np.empty(CH, dtype=np.float32)
        dd_[0] = -gates[0]
        dd_[1:] = -2.0 * gamma_
        fp8_ok = (
            np.abs(x).max() < 200.0
            and np.all(dd_.astype(f8).astype(np.float32) == dd_)
            and np.all(np.abs(dd_) <= 240.0)
            and np.all(gates.astype(f8).astype(np.float32) == gates)
        )
        if not fp8_ok:
            return _fallback_numpy(x, Wi, Wh, Wo, gates, l1, l2)
        nc = _get_module("fp8")
        d1_ = np.tile(np.tile(dd_, BS)[None, :], (128, 1)).astype(np.float32)
        in_maps = _prep_fp8_in_maps(x, gates, Wi_b.astype(np.float32),
                                    Wo_b.astype(np.float32), d1_)
        res = run_bass_kernel_spmd(nc, in_maps, core_ids=list(range(NCORES)))
        LAST_RESULTS = res
        out = np.empty((T, B, OUT), dtype=np.float32)
        inv_g = (1.0 / gamma_).astype(np.float32)
        for c in range(NCORES):
            ot = res.results[c]["outt"].reshape(OUT, BS, CH)[:, :, 1:]
            out[:, c * BS : (c + 1) * BS, :] = (
                -ot.transpose(2, 1, 0) * inv_g[:, None, None]
                - colsum[None, None, :]
            )
        return out

    nc = _get_module(mode)

    # augmented, transposed input-weights: [KAUG, H]
    wi_aug = np.empty((KAUG, H), dtype=np.float32)
    wi_aug[:IN] = Wi_b.T
    wi_aug[IN] = -1.0                       # g row
    wi_aug[IN + 1] = 1.0                    # reset row
    wo_arr = np.ascontiguousarray(Wo_b.T).astype(ml_dtypes.bfloat16)  # [H, OUT]

    # d1 per chain column: c=0 -> -g_1 ; c=1..63 -> -2*g_{c+1} ; c=64 -> -2
    gamma = np.empty(T, dtype=np.float32)   # scale for output recovery
    gamma[: T - 1] = gates[1:]
    gamma[T - 1] = 1.0
    dd = np.empty(CH, dtype=np.float32)
    dd[0] = -gates[0]
    dd[1:] = -2.0 * gamma
    d1 = np.tile(np.tile(dd, NB)[None, :], (128, 1)).astype(np.float32)

    in_maps = _prep_in_maps(x, gates, wi_aug, wo_arr, d1, mode)
    res = run_bass_kernel_spmd(nc, in_maps, core_ids=list(range(NCORES)))
    LAST_RESULTS = res

    out = np.empty((T, B, OUT), dtype=np.float32)
    inv_gamma = (1.0 / gamma).astype(np.float32)        # [T]
    for c in range(NCORES):
        ot = res.results[c]["outt"].reshape(OUT, BS, T)
        # out[t, b, o] = -ot[o, b, t]/gamma[t] - colsum[o]
        out[:, c * BS : (c + 1) * BS, :] = (
            -ot.transpose(2, 1, 0) * inv_gamma[:, None, None]
            - colsum[None, None, :]
        )
    return out

